# revision 1
# baseline (speedup 1.0000x reference)
"""BumpX pooling kernel for Trainium2 (8 NeuronCores, data-parallel over batch).

Math (per batch b, row l, position i, with a = aa[b,l,i], d = |j - i|):
    arg_d   = (d^2 - a^2) / (6a + 9)
    mask_d  = sigmoid(1/softplus(arg_d) - 1/softplus(1-arg_d))
    out[i]  = sum_d mask_d * (x[i-d] + x[i+d]) / sum_d mask_d * n_valid(i,d)

mask_d underflows to exactly 0 in fp32 for d >= 8 (for all a in [0,1)), so only
diagonals d = 0..7 are computed.

This build's ACT tables have no softplus/divide, and custom-DVE ISA ops don't
compile, so everything transcendental is composed from Exp/Ln (one ACT table
set, zero set switches):
    rden = Exp(-Ln(6a+9)) = 1/(6a+9)
    e1  = Exp(arg);  ecat = [e1 | e1 + (e-1)]           (DVE writes upper half)
    spc = Ln(ecat + 1) = [softplus(arg) | Ln(e1 + e)]
    sp2 = Ln(e1 + e) - arg = softplus(1 - arg)           (DVE, in place)
    rc  = Exp(-Ln(spc)) = [r1 | r2] = [1/sp1 | 1/sp2]
    ndf = min(r2, 43) - r1                               (clamp keeps Exp(ndf)
                                                          in the Ln table range)
    m   = Exp(-Ln(Exp(ndf) + 1)) = sigmoid(r1 - r2)

The d-stack is processed in two halves (d 0..3 / 4..7) software-pipelined
across ACT (transcendental chain), DVE (elementwise/reduces), and GpSimd
(shift-sums, mask*value products).  DMA issue is split between SP and the
otherwise-idle PE sequencer (descriptor generation costs ~0.7us per DMA).

Layout per core: partition p = c*16 + l (c = chunk of 128 positions, l = row);
stacks are (128, k=128, d=8) k-major so the d-reduction is contiguous.
Chunks c=0 / c=7 (the only ones with row-edge effects) sit on partition
ranges [0:16) / [112:128), handled with 32-partition-aligned edge ops.
"""

import numpy as np

import concourse.bass as bass
import concourse.mybir as mybir
from concourse.bass_utils import run_bass_kernel_spmd

F32 = mybir.dt.float32
L, F = 16, 1024
NC_COUNT = 8
W = 7          # max diagonal distance
ND = W + 1     # number of diagonals (d = 0..7)
HD = ND // 2   # half-stack depth
HALO = 8
XW = F // 8    # 128 positions per chunk
NCH = F // XW  # 8 chunks
E_CONST = float(np.exp(np.float64(1.0)))


class _FastBass(bass.Bass):
    """Skip the constructor's all-engine barrier (~3us): we never read the
    framework's const APs (all ACT biases are explicit tiles)."""

    def all_engine_barrier(self, *, sem_only: bool = False):
        if not getattr(self, "_init_barrier_skipped", False):
            self._init_barrier_skipped = True
            return
        return super().all_engine_barrier(sem_only=sem_only)


def _const_inputs():
    dsq = np.arange(ND, dtype=np.float32) ** 2                      # (8,)
    d = np.arange(ND)[None, :]
    k = np.arange(ND)[:, None]
    ec0 = (d > k).astype(np.float32)                                # (8k,8d) left
    ec7 = ((d + k) > W).astype(np.float32)                          # (8k,8d) right
    z = np.zeros_like(ec0)
    # edge ops use 32-partition slices covering chunks [0,1] / [6,7]; the
    # non-edge chunk gets a zero mask
    ec0e = np.stack([ec0, z])                                       # (2,8,8)
    ec7e = np.stack([z, ec7])                                       # (2,8,8)
    return dsq, ec0e, ec7e


def build_bass():
    nc = _FastBass("TRN2", debug=False)

    xpad = nc.dram_tensor("xpad", [L, F + 2 * HALO], F32, kind="ExternalInput").ap()
    aa = nc.dram_tensor("aa", [L, F], F32, kind="ExternalInput").ap()
    dsq_d = nc.dram_tensor("dsq", [ND], F32, kind="ExternalInput").ap()
    ec0_d = nc.dram_tensor("ec0", [2, ND, ND], F32, kind="ExternalInput").ap()
    ec7_d = nc.dram_tensor("ec7", [2, ND, ND], F32, kind="ExternalInput").ap()
    out = nc.dram_tensor("out", [L, F], F32, kind="ExternalOutput").ap()

    def sb(name, shape):
        return nc.alloc_sbuf_tensor(name, shape, F32).ap()

    XH = sb("XH", [128, XW + 2 * HALO])    # x with halo
    A = sb("A", [128, XW])
    DSQ = sb("DSQ", [128, ND])
    EC = sb("EC", [128, ND, ND])           # [p, k, d]: 0:32 left, 96:128 right
    CB0 = sb("CB0", [128, 1])              # 0.0   (ACT bias tiles)
    CB1 = sb("CB1", [128, 1])              # 1.0
    den6 = sb("den6", [128, XW])
    lden = sb("lden", [128, XW])
    lden2 = sb("lden2", [128, XW])
    rden = sb("rden", [128, XW])
    asq = sb("asq", [128, XW])
    arg = sb("arg", [128, XW, ND])         # k-major stacks
    E2 = sb("E2", [128, 2, XW, ND])        # [e1 | e1 + (e-1)]
    SPC = sb("SPC", [128, 2, XW, ND])      # [sp1 | Ln(e1+e) -> sp2]
    LC = sb("LC", [128, 2, XW, ND])
    RC = sb("RC", [128, 2, XW, ND])        # [r1 | r2]
    ndf = sb("ndf", [128, XW, ND])
    em = sb("em", [128, XW, ND])
    lm = sb("lm", [128, XW, ND])
    m = sb("m", [128, XW, ND])
    xs = sb("xs", [128, XW, ND])
    mp = sb("mp", [128, XW, ND])
    numA = sb("numA", [128, XW])
    numB = sb("numB", [128, XW])
    numf = sb("numf", [128, XW])
    SA = sb("SA", [128, XW])
    SB = sb("SB", [128, XW])
    D1 = sb("D1", [128, XW])
    den = sb("den", [128, XW])
    rdn = sb("rdn", [128, XW])
    et = sb("et", [128, ND, ND])
    ered = sb("ered", [128, ND])
    ered2 = sb("ered2", [128, ND])
    warm = sb("warm", [128, 1])
    O = sb("O", [128, XW])

    # DRAM-side access patterns with partition p = c*16 + l
    xh_src = bass.AP(tensor=xpad.tensor, offset=0,
                     ap=[[XW, NCH], [F + 2 * HALO, L], [1, XW + 2 * HALO]])
    aa_src = bass.AP(tensor=aa.tensor, offset=0,
                     ap=[[XW, NCH], [F, L], [1, XW]])
    dsq_src = bass.AP(tensor=dsq_d.tensor, offset=0, ap=[[0, 128], [1, ND]])
    ec0_src = bass.AP(tensor=ec0_d.tensor, offset=0,
                      ap=[[ND * ND, 2], [0, 16], [ND, ND], [1, ND]])
    ec7_src = bass.AP(tensor=ec7_d.tensor, offset=0,
                      ap=[[ND * ND, 2], [0, 16], [ND, ND], [1, ND]])
    out_dst0 = bass.AP(tensor=out.tensor, offset=0,
                       ap=[[XW, NCH // 2], [F, L], [1, XW]])
    out_dst1 = bass.AP(tensor=out.tensor, offset=(NCH // 2) * XW,
                       ap=[[XW, NCH // 2], [F, L], [1, XW]])

    AL = mybir.AluOpType
    AF = mybir.ActivationFunctionType

    def half(t, h):
        """d-half slice of a (128, XW, ND) stack."""
        return t[:, :, h * HD:(h + 1) * HD]

    def phalf(t, h):
        """d-half slice of a (128, 2, XW, ND) pair stack (4D AP)."""
        return t[:, :, :, h * HD:(h + 1) * HD]

    class Eng:
        """Engine op wrapper with minimal-dependency waits.

        Engines issue and COMPLETE instructions in order, but a later
        instruction's reads can start before an earlier one's writes land, so
        every data hazard needs a semaphore wait.  Each op incs the engine's
        chain sem on completion; `after=k` waits for the first k chained ops
        (completions are in order, so sem >= k  <=>  ops 1..k done).
        Redundant waits (value already awaited) are skipped."""

        def __init__(self, eng, sem):
            self.eng, self.sem, self.n = eng, sem, 0
            self.waited = {}

        def wait(self, sem, val):
            key = id(sem)
            if self.waited.get(key, -1) < val:
                self.eng.wait_ge(sem, val)
                self.waited[key] = val

        def op(self, make_inst, after=0, waits=()):
            for sem, val in waits:
                self.wait(sem, val)
            if after:
                self.wait(self.sem, after)
            inst = make_inst()
            inst.then_inc(self.sem, 1)
            self.n += 1
            assert self.n >= after
            return inst

    with (
        nc.Block(no_gpsimd_drain=True) as block,
        nc.semaphore("s_a") as s_a,
        nc.semaphore("s_x") as s_x,
        nc.semaphore("s_dsq") as s_dsq,
        nc.semaphore("s_c") as s_c,
        nc.semaphore("s_fin") as s_fin,
        nc.semaphore("s_v") as s_v,      # DVE chain
        nc.semaphore("s_t") as s_t,      # ACT chain
        nc.semaphore("s_g") as s_g,      # GPSIMD chain
    ):
        # chain-count milestones (asserted in the bodies)
        V_DEN6 = 1
        V_ARG = (4, 6)
        V_E1B = (7, 8)
        V_SP2 = (9, 10)
        V_NDF = (11, 13)
        V_DENF = 26
        V_OUT = 30
        T_RDEN = 3
        T_E1 = (4, 5)
        T_SPC = (6, 7)
        T_RC = (9, 14)
        T_M = (13, 17)
        G_CB = 3
        G_DSQ = 11
        G_XS = (15, 19)
        G_ETA = 21
        G_ETB = 23
        T_RDN2 = 19

        @block.sync
        def _(sync: bass.BassEngine):
            sync.dma_start(out=XH, in_=xh_src).then_inc(s_x, 16)
            sync.dma_start(out=EC[0:32], in_=ec0_src).then_inc(s_c, 16)
            sync.dma_start(out=EC[96:128], in_=ec7_src).then_inc(s_c, 16)
            sync.wait_ge(s_v, V_OUT)
            sync.dma_start(out=out_dst0, in_=O[0:64]).then_inc(s_fin, 16)
            sync.wait_ge(s_fin, 32)

        @block.gpsimd
        def _(g: bass.BassEngine):
            e = Eng(g, s_g)
            e.op(lambda: g.memset(CB0, 0.0))
            e.op(lambda: g.memset(CB1, 1.0))
            e.op(lambda: g.memset(warm, 1.0))
            assert e.n == G_CB, e.n
            # build DSQ = d^2 on-chip (no DMA dependency for the arg stage)
            for d in range(ND):
                e.op(lambda d=d: g.memset(DSQ[:, d:d + 1], float(d * d)))
            assert e.n == G_DSQ, e.n
            # xs shift-sums, delayed past DVE's arg phase (GpSimd shares SBUF
            # ports with DVE; running them concurrently slows DVE ~2x)
            for d in range(ND):
                if d == 0:
                    e.op(lambda: g.tensor_copy(xs[:, :, 0],
                                               XH[:, HALO:HALO + XW]),
                         waits=((s_x, 16), (s_v, V_ARG[1])))
                else:
                    e.op(lambda d=d: g.tensor_tensor(
                        xs[:, :, d], XH[:, HALO - d:HALO - d + XW],
                        XH[:, HALO + d:HALO + d + XW], op=AL.add))
            assert e.n == G_XS[1], e.n
            # A-half edge products (DVE is busy with its A tail then)
            e.op(lambda: g.tensor_tensor(et[0:32, :, 0:HD],
                                         m[0:32, 0:ND, 0:HD],
                                         EC[0:32, :, 0:HD], op=AL.mult),
                 waits=((s_t, T_M[0]), (s_c, 32)))
            e.op(lambda: g.tensor_tensor(et[96:128, :, 0:HD],
                                         m[96:128, XW - ND:XW, 0:HD],
                                         EC[96:128, :, 0:HD], op=AL.mult))
            assert e.n == G_ETA, e.n
            # B-half edge products as soon as mB lands (DVE then only reduces)
            e.op(lambda: g.tensor_tensor(et[0:32, :, HD:ND],
                                         m[0:32, 0:ND, HD:ND],
                                         EC[0:32, :, HD:ND], op=AL.mult),
                 waits=((s_t, T_M[1]),))
            e.op(lambda: g.tensor_tensor(et[96:128, :, HD:ND],
                                         m[96:128, XW - ND:XW, HD:ND],
                                         EC[96:128, :, HD:ND], op=AL.mult))
            assert e.n == G_ETB, e.n

        @block.scalar
        def _(act: bass.BassEngine):
            e = Eng(act, s_t)
            # ACT issues the critical-path aa DMA first thing (descriptor
            # generation costs ~0.7us per DMA per sequencer, so it is split
            # between ACT and SP)
            act.dma_start(out=A, in_=aa_src).then_inc(s_a, 16)
            # 1: warm the exp/ln table set while DMAs run
            e.op(lambda: act.activation(warm, warm, AF.Exp, bias=CB0),
                 waits=((s_g, G_CB),))
            # 2,3: rden = 1/(6a+9) = Exp(-Ln(den6))
            e.op(lambda: act.activation(lden, den6, AF.Ln, bias=CB0),
                 waits=((s_v, V_DEN6),))
            e.op(lambda: act.activation(rden, lden, AF.Exp,
                                        bias=CB0, scale=-1.0), after=2)
            assert e.n == T_RDEN, e.n
            # 4,5: e1 = Exp(arg)
            for h in range(2):
                e.op(lambda h=h: act.activation(phalf(E2, h)[:, 0],
                                                half(arg, h), AF.Exp,
                                                bias=CB0),
                     waits=((s_v, V_ARG[h]),))
            assert e.n == T_E1[1], e.n
            # 6,7: spc = Ln(ecat + 1) = [sp1 | Ln(e1+e)]
            for h in range(2):
                e.op(lambda h=h: act.activation(phalf(SPC, h), phalf(E2, h),
                                                AF.Ln, bias=CB1),
                     after=T_E1[h], waits=((s_v, V_E1B[h]),))
            assert e.n == T_SPC[1], e.n
            # 8,9: lcA, rcA
            e.op(lambda: act.activation(phalf(LC, 0), phalf(SPC, 0),
                                        AF.Ln, bias=CB0),
                 after=T_SPC[0], waits=((s_v, V_SP2[0]),))
            e.op(lambda: act.activation(phalf(RC, 0), phalf(LC, 0),
                                        AF.Exp, bias=CB0, scale=-1.0),
                 after=8)
            assert e.n == T_RC[0], e.n
            # 10: lcB (fills the gap while DVE computes ndfA)
            e.op(lambda: act.activation(phalf(LC, 1), phalf(SPC, 1),
                                        AF.Ln, bias=CB0),
                 after=T_SPC[1], waits=((s_v, V_SP2[1]),))
            # 11-13: trio A -> mA as early as possible
            e.op(lambda: act.activation(half(em, 0), half(ndf, 0),
                                        AF.Exp, bias=CB0),
                 waits=((s_v, V_NDF[0]),))
            e.op(lambda: act.activation(half(lm, 0), half(em, 0),
                                        AF.Ln, bias=CB1), after=11)
            e.op(lambda: act.activation(half(m, 0), half(lm, 0),
                                        AF.Exp, bias=CB0, scale=-1.0),
                 after=12)
            assert e.n == T_M[0], e.n
            # 14: rcB
            e.op(lambda: act.activation(phalf(RC, 1), phalf(LC, 1),
                                        AF.Exp, bias=CB0, scale=-1.0),
                 after=10)
            assert e.n == T_RC[1], e.n
            # 15-17: trio B
            e.op(lambda: act.activation(half(em, 1), half(ndf, 1),
                                        AF.Exp, bias=CB0),
                 waits=((s_v, V_NDF[1]),))
            e.op(lambda: act.activation(half(lm, 1), half(em, 1),
                                        AF.Ln, bias=CB1), after=15)
            e.op(lambda: act.activation(half(m, 1), half(lm, 1),
                                        AF.Exp, bias=CB0, scale=-1.0),
                 after=16)
            assert e.n == T_M[1], e.n
            # 18,19: rdn = 1/den = Exp(-Ln(den)), overlapped with DVE's
            # numerator work
            e.op(lambda: act.activation(lden2, den, AF.Ln, bias=CB0),
                 waits=((s_v, V_DENF),))
            e.op(lambda: act.activation(rdn, lden2, AF.Exp,
                                        bias=CB0, scale=-1.0), after=18)
            assert e.n == T_RDN2, e.n
            # second half of the output store, issued in parallel with SP's
            act.wait_ge(s_v, V_OUT)
            act.dma_start(out=out_dst1, in_=O[64:128]).then_inc(s_fin, 16)

        @block.vector
        def _(v: bass.BassEngine):
            e = Eng(v, s_v)
            dsq_b = DSQ.unsqueeze(1).broadcast_to([128, XW, ND])
            asq_b = asq.unsqueeze(2).broadcast_to([128, XW, ND])
            rden_b = rden.unsqueeze(2).broadcast_to([128, XW, ND])
            # 1,2: prologue
            e.op(lambda: v.tensor_scalar(den6, A, 6.0, 9.0,
                                         op0=AL.mult, op1=AL.add),
                 waits=((s_a, 16),))
            e.op(lambda: v.tensor_tensor(asq, A, A, op=AL.mult))
            # 3-6: arg halves
            for h in range(2):
                e.op(lambda h=h: v.tensor_tensor(half(arg, h), half(dsq_b, h),
                                                 half(asq_b, h),
                                                 op=AL.subtract),
                     after=2, waits=((s_g, G_DSQ),))
                e.op(lambda h=h: v.tensor_tensor(half(arg, h), half(arg, h),
                                                 half(rden_b, h), op=AL.mult),
                     after=e.n, waits=((s_t, T_RDEN),))
                assert e.n == V_ARG[h], e.n
            # 7,8: ecat upper half = e1 + (e-1)
            for h in range(2):
                e.op(lambda h=h: v.tensor_scalar_add(
                    phalf(E2, h)[:, 1], phalf(E2, h)[:, 0], E_CONST - 1.0),
                     waits=((s_t, T_E1[h]),))
                assert e.n == V_E1B[h], e.n
            # 9,10: sp2 = Ln(e1+e) - arg, in place
            for h in range(2):
                e.op(lambda h=h: v.tensor_tensor(
                    phalf(SPC, h)[:, 1], phalf(SPC, h)[:, 1], half(arg, h),
                    op=AL.subtract),
                     after=V_ARG[h], waits=((s_t, T_SPC[h]),))
                assert e.n == V_SP2[h], e.n
            # 11: ndfA = min(r2, 43) - r1
            e.op(lambda: v.scalar_tensor_tensor(
                half(ndf, 0), phalf(RC, 0)[:, 1], 43.0, phalf(RC, 0)[:, 0],
                op0=AL.min, op1=AL.subtract),
                 waits=((s_t, T_RC[0]),))
            assert e.n == V_NDF[0], e.n
            # 12: SA (mA ready)
            e.op(lambda: v.tensor_reduce(SA, half(m, 0),
                                         axis=mybir.AxisListType.X,
                                         op=AL.add),
                 waits=((s_t, T_M[0]),))
            # 13: ndfB (rcB ready; unblocks ACT trio B)
            e.op(lambda: v.scalar_tensor_tensor(
                half(ndf, 1), phalf(RC, 1)[:, 1], 43.0, phalf(RC, 1)[:, 0],
                op0=AL.min, op1=AL.subtract),
                 waits=((s_t, T_RC[1]),))
            assert e.n == V_NDF[1], e.n
            # 14-20: A-half tail, hidden under ACT's trio-B
            e.op(lambda: v.tensor_tensor(half(mp, 0), half(m, 0), half(xs, 0),
                                         op=AL.mult),
                 waits=((s_g, G_XS[0]),))                        # 14
            e.op(lambda: v.tensor_reduce(numA, half(mp, 0),
                                         axis=mybir.AxisListType.X,
                                         op=AL.add), after=14)   # 15
            e.op(lambda: v.scalar_tensor_tensor(D1, SA, 2.0, m[:, :, 0],
                                                op0=AL.mult, op1=AL.subtract),
                 after=12)                                       # 16
            e.op(lambda: v.tensor_reduce(ered[0:32], et[0:32, :, 0:HD],
                                         axis=mybir.AxisListType.X,
                                         op=AL.add),
                 waits=((s_g, G_ETA),))                          # 17
            e.op(lambda: v.tensor_reduce(ered[96:128], et[96:128, :, 0:HD],
                                         axis=mybir.AxisListType.X,
                                         op=AL.add))             # 18
            e.op(lambda: v.tensor_tensor(D1[0:32, 0:ND], D1[0:32, 0:ND],
                                         ered[0:32], op=AL.subtract),
                 after=17)                                       # 19
            e.op(lambda: v.tensor_tensor(D1[96:128, XW - ND:XW],
                                         D1[96:128, XW - ND:XW],
                                         ered[96:128], op=AL.subtract),
                 after=18)                                       # 20
            # 21-28: denominator path (feeds ACT's reciprocal)
            e.op(lambda: v.tensor_reduce(SB, half(m, 1),
                                         axis=mybir.AxisListType.X,
                                         op=AL.add),
                 waits=((s_t, T_M[1]),))                         # 21
            e.op(lambda: v.scalar_tensor_tensor(den, SB, 2.0, D1,
                                                op0=AL.mult, op1=AL.add),
                 after=21)                                       # 22
            e.op(lambda: v.tensor_reduce(ered2[0:32], et[0:32, :, HD:ND],
                                         axis=mybir.AxisListType.X,
                                         op=AL.add),
                 waits=((s_g, G_ETB),))                          # 23
            e.op(lambda: v.tensor_reduce(ered2[96:128], et[96:128, :, HD:ND],
                                         axis=mybir.AxisListType.X,
                                         op=AL.add))             # 24
            e.op(lambda: v.tensor_tensor(den[0:32, 0:ND], den[0:32, 0:ND],
                                         ered2[0:32], op=AL.subtract),
                 after=23)                                       # 25
            e.op(lambda: v.tensor_tensor(den[96:128, XW - ND:XW],
                                         den[96:128, XW - ND:XW],
                                         ered2[96:128], op=AL.subtract),
                 after=24)                                       # 26
            assert e.n == V_DENF, e.n
            # 27-30: numerator path overlaps ACT's reciprocal
            e.op(lambda: v.tensor_tensor(half(mp, 1), half(m, 1), half(xs, 1),
                                         op=AL.mult),
                 waits=((s_g, G_XS[1]),))                        # 27
            e.op(lambda: v.tensor_reduce(numB, half(mp, 1),
                                         axis=mybir.AxisListType.X,
                                         op=AL.add), after=27)   # 28
            e.op(lambda: v.tensor_tensor(numf, numA, numB, op=AL.add),
                 after=28)                                       # 29
            e.op(lambda: v.tensor_tensor(O, numf, rdn, op=AL.mult),
                 after=29, waits=((s_t, T_RDN2),))               # 30
            assert e.n == V_OUT, e.n

    return nc


_NC_CACHE = None


def _get_nc():
    global _NC_CACHE
    if _NC_CACHE is None:
        _NC_CACHE = build_bass()
    return _NC_CACHE


def make_in_maps(x, aa):
    x = np.asarray(x, dtype=np.float32)
    aa = np.asarray(aa, dtype=np.float32)
    dsq, ec0, ec7 = _const_inputs()
    in_maps = []
    for b in range(NC_COUNT):
        xp = np.pad(np.ascontiguousarray(x[b], dtype=np.float32),
                    ((0, 0), (HALO, HALO)))
        in_maps.append({
            "xpad": xp,
            "aa": np.ascontiguousarray(aa[b], dtype=np.float32),
            "dsq": dsq, "ec0": ec0, "ec7": ec7,
        })
    return in_maps


def kernel(x, aa):
    nc = _get_nc()
    res = run_bass_kernel_spmd(nc, make_in_maps(x, aa),
                               core_ids=list(range(NC_COUNT)))
    return np.stack([res.results[b]["out"] for b in range(NC_COUNT)], axis=0)



# revision 4
# speedup vs baseline: 1.2213x; 1.2213x over previous
"""BumpX pooling kernel for Trainium2 (8 NeuronCores, data-parallel over batch).

Math (per batch b, row l, position i, a = aa[b,l,i], d = |j - i|):
    arg_d = (d^2 - a^2) / (6a + 9)
    m_d   = 1 - gg(arg_d)        (the bump mask; underflows for d >= 7)
    out_i = sum_d m_d (x[i-d] + x[i+d]) / sum_d m_d n_valid(i, d)

Approximations (rel tolerance 2e-2; this lands ~5e-3):
  - m(t) = exp(-exp(g(t))), g fitted by a degree-4 polynomial over
    t in [-0.08, 4.01] (the full arg range for d <= 6); d=7 dropped.
  - mask/tap stacks held in bf16 (2x DVE rate on packed tensor_tensor);
    reductions accumulate in fp32.

Per-element chain: arg (2 DVE ops) -> monic Horner q (3 fused DVE ops)
-> E1 = Exp(c4*q + c0) (ACT) -> m = Exp(-E1) (ACT). The d=0 mask is
halved in place so the symmetric-tap stack xs_d = x[i-d] + x[i+d]
(ONE DVE op via a +/-1-stride view pair, xs_0 = 2x) gives
num = reduce(m*xs) and den = 2*reduce(m); row-edge taps are removed with
masked products on GpSimd + 32-partition fixups.

The stacks are processed in two position halves software-pipelined
across DVE and ACT: while ACT runs Exp/Exp on half 0, DVE computes
arg/poly of half 1; each half's output columns are DMAed as soon as its
numerator/denominator finish (SP stores half 0, ACT half 1).

Layout per core (core = batch): partition p = c*16 + l (c = chunk of 128
positions, l = row); stacks are (128, 128, 7) with d innermost so the
d-reduction is a contiguous X-reduce. Inputs arrive as two 2D DMAs:
bf16 [x-halo(140) | edge-masks(49)] per partition, and fp32 aa (128x128).
"""

import numpy as np

import concourse.bass as bass
import concourse.mybir as mybir
from concourse.bass_utils import run_bass_kernel_spmd

F32 = mybir.dt.float32
BF16 = mybir.dt.bfloat16
L, F = 16, 1024
NC_COUNT = 8
W = 6          # max diagonal distance kept
ND = W + 1     # stack depth (d = 0..6)
HALO = W
XW = 128       # positions per chunk
HW_ = XW // 2  # position half width
NCH = F // XW  # 8 chunks
XROW = XW + 2 * HALO           # 140
PITCH = XROW + ND * ND         # 189: [x-halo | EC masks]

# g(t) = ln(-ln(m(t))) degree-4 weighted fit over t in [-0.08, 4.01]
GC = (-0.029456496983506418, 0.18552920622532633, -0.3712109527981173,
      1.1947827839859845, -0.8925694191796499)
C4, C3, C2, C1, C0 = GC
U3, U2, U1 = C3 / C4, C2 / C4, C1 / C4


class _FastBass(bass.Bass):
    """Skip the constructor's all-engine barrier (~3us): we never read the
    framework's const APs (all ACT biases are explicit tiles)."""

    def all_engine_barrier(self, *, sem_only: bool = False):
        if not getattr(self, "_init_barrier_skipped", False):
            self._init_barrier_skipped = True
            return
        return super().all_engine_barrier(sem_only=sem_only)


def build_bass():
    nc = _FastBass("TRN2", debug=False)

    xe_d = nc.dram_tensor("xe", [128, PITCH], BF16, kind="ExternalInput").ap()
    aa_d = nc.dram_tensor("aa", [128, XW], F32, kind="ExternalInput").ap()
    out_d = nc.dram_tensor("out", [128, XW], F32, kind="ExternalOutput").ap()

    def sb(name, shape, dt=F32):
        return nc.alloc_sbuf_tensor(name, shape, dt).ap()

    XE = sb("XE", [128, PITCH], BF16)  # [x-halo(140) | EC(7,7)(49)]
    A = sb("A", [128, XW])
    DSQ = sb("DSQ", [128, ND])
    CB0 = sb("CB0", [128, 1])          # 0.0 (ACT bias)
    CG0 = sb("CG0", [128, 1])          # C0  (ACT bias for E1)
    WRM = sb("WRM", [128, 1])
    den6 = sb("den6", [128, XW])
    lden = sb("lden", [128, XW])
    rden = sb("rden", [128, XW])
    asq = sb("asq", [128, XW])
    arg = sb("arg", [128, XW, ND], BF16)
    q = sb("q", [128, XW, ND], BF16)
    E1 = sb("E1", [128, XW, ND])
    m = sb("m", [128, XW, ND], BF16)
    xs = sb("xs", [128, XW, ND], BF16)
    mp = sb("mp", [128, XW, ND], BF16)
    S = sb("S", [128, XW])
    den = sb("den", [128, XW])
    lden2 = sb("lden2", [128, XW])
    rdn = sb("rdn", [128, XW])
    num = sb("num", [128, XW])
    O = sb("O", [128, XW])
    et = sb("et", [128, ND, ND], BF16)  # edge products ([0:32] / [96:128])
    ered = sb("ered", [128, ND])

    # EC view: XE[:, 140:189] seen as (128, 7, 7) [k, d]
    EC = bass.AP(tensor=XE.tensor, offset=XROW,
                 ap=[[PITCH, 128], [ND, ND], [1, ND]])
    # xs operand views: elem (p, i, d) -> XE[p, HALO + i -/+ d]
    xm_v = bass.AP(tensor=XE.tensor, offset=HALO,
                   ap=[[PITCH, 128], [1, XW], [-1, ND]])
    xp_v = bass.AP(tensor=XE.tensor, offset=HALO,
                   ap=[[PITCH, 128], [1, XW], [1, ND]])

    AL = mybir.AluOpType
    AF = mybir.ActivationFunctionType

    def h(t, k, extra=None):
        """Column-half slice of a (128, XW, ...) or (128, XW) AP."""
        sl = t[:, k * HW_:(k + 1) * HW_]
        return sl

    class Eng:
        """Engine op wrapper with minimal-dependency waits (see baseline)."""

        def __init__(self, eng, sem):
            self.eng, self.sem, self.n = eng, sem, 0
            self.waited = {}

        def wait(self, sem, val):
            key = id(sem)
            if self.waited.get(key, -1) < val:
                self.eng.wait_ge(sem, val)
                self.waited[key] = val

        def op(self, make_inst, after=0, waits=()):
            for sem, val in waits:
                self.wait(sem, val)
            if after:
                self.wait(self.sem, after)
            inst = make_inst()
            inst.then_inc(self.sem, 1)
            self.n += 1
            assert self.n >= after
            return inst

    with (
        nc.Block(no_gpsimd_drain=True) as block,
        nc.semaphore("s_a") as s_a,
        nc.semaphore("s_x") as s_x,
        nc.semaphore("s_fin") as s_fin,
        nc.semaphore("s_v") as s_v,      # DVE chain
        nc.semaphore("s_t") as s_t,      # ACT chain
        nc.semaphore("s_g") as s_g,      # GPSIMD chain
    ):
        # chain-count milestones
        G_CB = 2
        G_DSQ = 9
        G_ETL = 10
        G_ETR = 11
        V_POLY = (8, 13)
        V_DEN = (18, 26)
        V_OUT = (21, 29)
        T_RDEN = 3
        T_M = (5, 7)
        T_RDN = (9, 11)

        @block.sync
        def _(sync: bass.BassEngine):
            sync.dma_start(out=XE, in_=xe_d).then_inc(s_x, 16)
            sync.wait_ge(s_v, V_OUT[0])
            sync.dma_start(out=out_d[:, 0:HW_],
                           in_=O[:, 0:HW_]).then_inc(s_fin, 16)
            sync.wait_ge(s_fin, 32)

        @block.gpsimd
        def _(g: bass.BassEngine):
            e = Eng(g, s_g)
            e.op(lambda: g.memset(CB0, 0.0))
            e.op(lambda: g.memset(CG0, float(C0)))
            assert e.n == G_CB, e.n
            for d in range(ND):
                e.op(lambda d=d: g.memset(DSQ[:, d:d + 1], float(d * d)))
            assert e.n == G_DSQ, e.n
            # edge products (d >= 1 only; the d=0 column is being halved)
            e.op(lambda: g.tensor_tensor(et[0:32, :, 1:ND],
                                         m[0:32, 0:ND, 1:ND],
                                         EC[0:32, :, 1:ND], op=AL.mult),
                 waits=((s_t, T_M[0]), (s_x, 16)))
            assert e.n == G_ETL, e.n
            e.op(lambda: g.tensor_tensor(et[96:128, :, 1:ND],
                                         m[96:128, XW - ND:XW, 1:ND],
                                         EC[96:128, :, 1:ND], op=AL.mult),
                 waits=((s_t, T_M[1]),))
            assert e.n == G_ETR, e.n

        @block.scalar
        def _(act: bass.BassEngine):
            e = Eng(act, s_t)
            act.dma_start(out=A, in_=aa_d).then_inc(s_a, 16)
            # 1: warm the exp/ln table set while DMAs run
            e.op(lambda: act.activation(WRM, CB0, AF.Exp, bias=CB0),
                 waits=((s_g, G_CB),))
            # 2,3: rden = 1/(6a+9) = Exp(-Ln(den6))
            e.op(lambda: act.activation(lden, den6, AF.Ln, bias=CB0),
                 waits=((s_v, 1),))
            e.op(lambda: act.activation(rden, lden, AF.Exp,
                                        bias=CB0, scale=-1.0), after=2)
            assert e.n == T_RDEN, e.n
            # 4-7: E1 = Exp(c4*q + c0); m = Exp(-E1), per half
            for k in range(2):
                e.op(lambda k=k: act.activation(h(E1, k), h(q, k), AF.Exp,
                                                bias=CG0, scale=float(C4)),
                     waits=((s_v, V_POLY[k]),))
                e.op(lambda k=k: act.activation(h(m, k), h(E1, k), AF.Exp,
                                                bias=CB0, scale=-1.0),
                     after=e.n)
                assert e.n == T_M[k], e.n
            # 8-11: rdn = 1/den per half
            for k in range(2):
                e.op(lambda k=k: act.activation(h(lden2, k), h(den, k),
                                                AF.Ln, bias=CB0),
                     waits=((s_v, V_DEN[k]),))
                e.op(lambda k=k: act.activation(h(rdn, k), h(lden2, k),
                                                AF.Exp, bias=CB0, scale=-1.0),
                     after=e.n)
                assert e.n == T_RDN[k], e.n
            act.wait_ge(s_v, V_OUT[1])
            act.dma_start(out=out_d[:, HW_:XW],
                          in_=O[:, HW_:XW]).then_inc(s_fin, 16)

        @block.vector
        def _(v: bass.BassEngine):
            e = Eng(v, s_v)
            dsq_b = DSQ.unsqueeze(1).broadcast_to([128, XW, ND])
            asq_b = asq.unsqueeze(2).broadcast_to([128, XW, ND])
            rden_b = rden.unsqueeze(2).broadcast_to([128, XW, ND])
            # 1,2: prologue (needs aa)
            e.op(lambda: v.tensor_scalar(den6, A, 6.0, 9.0,
                                         op0=AL.mult, op1=AL.add),
                 waits=((s_a, 16),))
            e.op(lambda: v.tensor_tensor(asq, A, A, op=AL.mult))
            # 3: symmetric tap sums (xs_0 = 2x; m_0 is halved to compensate)
            e.op(lambda: v.tensor_tensor(xs, xm_v, xp_v, op=AL.add),
                 waits=((s_x, 16),))
            # 4-13: arg + monic Horner, per half
            for k in range(2):
                e.op(lambda k=k: v.tensor_tensor(h(arg, k), h(dsq_b, k),
                                                 h(asq_b, k), op=AL.subtract),
                     after=2, waits=((s_g, G_DSQ),))
                e.op(lambda k=k: v.tensor_tensor(h(arg, k), h(arg, k),
                                                 h(rden_b, k), op=AL.mult),
                     after=e.n, waits=((s_t, T_RDEN),))
                e.op(lambda k=k: v.scalar_tensor_tensor(
                    h(q, k), h(arg, k), float(U3), h(arg, k),
                    op0=AL.add, op1=AL.mult), after=e.n)
                e.op(lambda k=k: v.scalar_tensor_tensor(
                    h(q, k), h(q, k), float(U2), h(arg, k),
                    op0=AL.add, op1=AL.mult), after=e.n)
                e.op(lambda k=k: v.scalar_tensor_tensor(
                    h(q, k), h(q, k), float(U1), h(arg, k),
                    op0=AL.add, op1=AL.mult), after=e.n)
                assert e.n == V_POLY[k], e.n
            # per-half tails
            for k in range(2):
                base = e.n
                # halve m_0; S = reduce(m); den = 2S
                e.op(lambda k=k: v.tensor_scalar(
                    m[:, k * HW_:(k + 1) * HW_, 0:1],
                    m[:, k * HW_:(k + 1) * HW_, 0:1], 0.5, 0.0,
                    op0=AL.mult, op1=AL.add),
                     waits=((s_t, T_M[k]),))
                e.op(lambda k=k: v.tensor_reduce(h(S, k), h(m, k),
                                                 axis=mybir.AxisListType.X,
                                                 op=AL.add), after=e.n)
                e.op(lambda k=k: v.tensor_scalar(h(den, k), h(S, k), 2.0, 0.0,
                                                 op0=AL.mult, op1=AL.add),
                     after=e.n)
                # edge fixups (left edge lives in half 0, right in half 1)
                if k == 0:
                    e.op(lambda: v.tensor_reduce(ered[0:32],
                                                 et[0:32, :, 1:ND],
                                                 axis=mybir.AxisListType.X,
                                                 op=AL.add),
                         waits=((s_g, G_ETL),))
                    e.op(lambda: v.tensor_tensor(den[0:32, 0:ND],
                                                 den[0:32, 0:ND],
                                                 ered[0:32], op=AL.subtract),
                         after=e.n)
                else:
                    e.op(lambda: v.tensor_reduce(ered[96:128],
                                                 et[96:128, :, 1:ND],
                                                 axis=mybir.AxisListType.X,
                                                 op=AL.add),
                         waits=((s_g, G_ETR),))
                    e.op(lambda: v.tensor_tensor(den[96:128, XW - ND:XW],
                                                 den[96:128, XW - ND:XW],
                                                 ered[96:128],
                                                 op=AL.subtract),
                         after=e.n)
                assert e.n == V_DEN[k], e.n
                # numerator + output
                e.op(lambda k=k: v.tensor_tensor(h(mp, k), h(m, k), h(xs, k),
                                                 op=AL.mult), after=base + 1)
                e.op(lambda k=k: v.tensor_reduce(h(num, k), h(mp, k),
                                                 axis=mybir.AxisListType.X,
                                                 op=AL.add), after=e.n)
                e.op(lambda k=k: v.tensor_tensor(h(O, k), h(num, k),
                                                 h(rdn, k), op=AL.mult),
                     after=e.n, waits=((s_t, T_RDN[k]),))
                assert e.n == V_OUT[k], e.n

    return nc


_NC_CACHE = None


def _get_nc():
    global _NC_CACHE
    if _NC_CACHE is None:
        _NC_CACHE = build_bass()
    return _NC_CACHE


def _ec_host():
    k = np.arange(ND)[:, None]
    d = np.arange(ND)[None, :]
    ec = np.zeros((128, ND, ND), np.float32)
    ec[0:16] = (d > k).astype(np.float32)
    ec[112:128] = ((d + k) > W).astype(np.float32)
    return ec.reshape(128, ND * ND)


def make_in_maps(x, aa):
    import ml_dtypes
    x = np.asarray(x, dtype=np.float32)
    aa = np.asarray(aa, dtype=np.float32)
    ec = _ec_host()
    in_maps = []
    for b in range(NC_COUNT):
        xp = np.pad(x[b], ((0, 0), (HALO, HALO)))   # (16, 1036)
        xe = np.empty((128, PITCH), np.float32)
        xh = np.stack([xp[:, c * XW:c * XW + XROW] for c in range(NCH)])
        xe[:, 0:XROW] = xh.reshape(128, XROW)
        xe[:, XROW:] = ec
        ah = np.stack([aa[b][:, c * XW:(c + 1) * XW] for c in range(NCH)])
        in_maps.append({"xe": xe.astype(ml_dtypes.bfloat16),
                        "aa": ah.reshape(128, XW).copy()})
    return in_maps


def gather_out(o):
    return np.asarray(o).reshape(NCH, L, XW).transpose(1, 0, 2).reshape(L, F)


def kernel(x, aa):
    nc = _get_nc()
    res = run_bass_kernel_spmd(nc, make_in_maps(x, aa),
                               core_ids=list(range(NC_COUNT)))
    return np.stack([gather_out(res.results[b]["out"])
                     for b in range(NC_COUNT)], axis=0)
